# revision 3
# baseline (speedup 1.0000x reference)
"""JacobiKAN layer on 8 TRN2 NeuronCores — data-parallel Bass/Tile kernel.

  reference: out = silu(LN(silu(x) @ W.T + einsum('bid,iod->bo', jacobi(tanh x), C)))
  x [8192, 1024], W [1024, 1024], C [1024, 1024, 9]; order-8 Jacobi (a=b=1).

Strategy
  - Shard the token dim B=8192 across 8 cores (1024 rows each); weights
    replicated.  No collectives.
  - Reformulate the Jacobi einsum in the monomial basis:
        y = sum_m t^m @ D_m,   D_m = sum_d M[d,m] C[:,:,d]
    (M = Jacobi->monomial change of basis, host-precomputed).  The m=0 term
    is x-independent: a bias row v = sum_i D_0[i,:], injected with a K=1
    ones-matmul.  Device computes t^2..t^8 with 3 ScalarE squares + 4
    VectorE mults per tile — half the elementwise cost of the 2-op-per-
    degree Jacobi recurrence.
  - All matmuls in fp32r (fp32 with 12-bit mantissa, 1 PE cycle/row at
    N=512 — bf16 speed).  End-to-end scale-relative error vs the fp32
    reference is ~2e-4 (basis term dominates z, so the monomial
    amplification stays subdominant).
  - PSUM holds half the per-core output: two o-half passes, 8 banks of
    [128b, 512o] each; z parked in SBUF between passes; LayerNorm+SiLU
    fused at the end (bn_stats/bn_aggr + one ScalarE Silu with per-row
    scale/bias).
"""
import os
import sys
from contextlib import ExitStack

import numpy as np

for _p in ("/opt/trn_rl_repo",):
    if _p not in sys.path and os.path.isdir(_p):
        sys.path.append(_p)

import concourse.bacc as bacc
import concourse.mybir as mybir
import concourse.tile as tile
from concourse.bass_utils import run_bass_kernel_spmd
from neuronxcc.starfish.support.dtype import (
    static_cast_fp32_to_fp32r,
    static_cast_fp32r_to_fp32,
)

F32 = mybir.dt.float32
F32R = mybir.dt.float32r
AF = mybir.ActivationFunctionType
ALU = mybir.AluOpType

N_CORES = 8
B_FULL, IN_F, OUT_F, ORDER = 8192, 1024, 1024, 8
B_CORE = B_FULL // N_CORES          # 1024 rows per core
LN_EPS = 1e-5
N_K = IN_F // 128                   # 8 contraction chunks
N_J = B_CORE // 128                 # 8 output row-tiles per core
N_OH = 2                            # two 512-wide o halves (PSUM capacity)


def _q(a):
    """Round to fp32r (12-bit mantissa) — what the PE array consumes."""
    return static_cast_fp32r_to_fp32(
        static_cast_fp32_to_fp32r(np.ascontiguousarray(a, np.float32)))


def _monomial_matrix():
    """M[d, m]: P_d^{(1,1)}(t) = sum_m M[d,m] t^m (reference recurrence)."""
    a = b = 1.0
    M = np.zeros((ORDER + 1, ORDER + 1))
    M[0, 0] = 1.0
    M[1, 1] = (a + b + 2.0) / 2.0
    M[1, 0] = (a - b) / 2.0
    for i in range(2, ORDER + 1):
        th_k = (2 * i + a + b) * (2 * i + a + b - 1) / (2 * i * (i + a + b))
        th_k1 = ((2 * i + a + b - 1) * (a * a - b * b)
                 / (2 * i * (i + a + b) * (2 * i + a + b - 2)))
        th_k2 = ((i + a - 1) * (i + b - 1) * (2 * i + a + b)
                 / (i * (i + a + b) * (2 * i + a + b - 2)))
        M[i, 1:] += th_k * M[i - 1, :-1]
        M[i, :] += th_k1 * M[i - 1, :]
        M[i, :] -= th_k2 * M[i - 2, :]
    return M


def _build_program(general_ln):
    nc = bacc.Bacc("TRN2", target_bir_lowering=False, debug=False)

    xt_d = nc.dram_tensor("xt", [IN_F, B_CORE], F32, kind="ExternalInput").ap()
    dm_d = nc.dram_tensor("dmono", [N_OH, N_K, ORDER, 128, 512], F32R,
                          kind="ExternalInput").ap()
    wt_d = nc.dram_tensor("wtp", [N_OH, N_K, 128, 512], F32R,
                          kind="ExternalInput").ap()
    v_d = nc.dram_tensor("vrow", [1, OUT_F], F32R, kind="ExternalInput").ap()
    one_d = nc.dram_tensor("onerow", [1, 128], F32R, kind="ExternalInput").ap()
    if general_ln:
        lnw_d = nc.dram_tensor("lnw", [1, OUT_F], F32, kind="ExternalInput").ap()
        lnb_d = nc.dram_tensor("lnb", [1, OUT_F], F32, kind="ExternalInput").ap()
    out_d = nc.dram_tensor("out", [B_CORE, OUT_F], F32,
                           kind="ExternalOutput").ap()

    with tile.TileContext(nc) as tc:
        with ExitStack() as ctx:
            const = ctx.enter_context(tc.tile_pool(name="const", bufs=1))
            xload = ctx.enter_context(tc.tile_pool(name="xload", bufs=2))
            acts = ctx.enter_context(tc.tile_pool(name="acts", bufs=1))
            zpark = ctx.enter_context(tc.tile_pool(name="zpark", bufs=1))
            pwp = ctx.enter_context(tc.tile_pool(name="pwp", bufs=2))
            dstr = ctx.enter_context(tc.tile_pool(name="dstr", bufs=2))
            outp = ctx.enter_context(tc.tile_pool(name="outp", bufs=2))
            stat = ctx.enter_context(tc.tile_pool(name="stat", bufs=4))
            psum = ctx.enter_context(tc.tile_pool(name="psum", bufs=1,
                                                  space="PSUM"))

            ones_t = const.tile([1, 128], F32R)
            nc.sync.dma_start(ones_t, one_d)
            eps_t = const.tile([128, 1], F32)
            nc.vector.memset(eps_t, LN_EPS)
            v_t = const.tile([1, OUT_F], F32R)
            nc.sync.dma_start(v_t, v_d)
            if general_ln:
                import concourse.bass as bass
                lnw_t = const.tile([128, OUT_F], F32)
                nc.sync.dma_start(lnw_t, bass.AP(
                    tensor=lnw_d.tensor, offset=lnw_d.offset,
                    ap=[[0, 128]] + list(lnw_d.ap[1:])))
                lnb_t = const.tile([128, OUT_F], F32)
                nc.sync.dma_start(lnb_t, bass.AP(
                    tensor=lnb_d.tensor, offset=lnb_d.offset,
                    ap=[[0, 128]] + list(lnb_d.ap[1:])))

            # T[k] = tanh(x.T chunk), SIL[k] = silu(x.T chunk); fp32r outs
            # (matmul lhsT producers must be fp32r-typed for the verifier).
            T = []
            SIL = []
            for k in range(N_K):
                xt_t = xload.tile([128, B_CORE], F32, name=f"xt_{k}",
                                  tag="xt")
                nc.sync.dma_start(xt_t, xt_d[128 * k:128 * (k + 1), :])
                t_t = acts.tile([128, B_CORE], F32R, name=f"tanh_{k}",
                                tag=f"tanh_{k}")
                nc.scalar.activation(t_t, xt_t, AF.Tanh)
                s_t = acts.tile([128, B_CORE], F32R, name=f"sil_{k}",
                                tag=f"sil_{k}")
                nc.scalar.activation(s_t, xt_t, AF.Silu)
                T.append(t_t)
                SIL.append(s_t)

            z = [zpark.tile([128, OUT_F], F32, name=f"z_{j}", tag=f"z_{j}")
                 for j in range(N_J)]

            for oh in range(N_OH):
                osl = slice(512 * oh, 512 * (oh + 1))
                ps = [psum.tile([128, 512], F32, name=f"ps_{oh}_{j}",
                                tag=f"ps_{j}") for j in range(N_J)]
                for k in range(N_K):
                    dm_t = dstr.tile([128, ORDER, 512], F32R,
                                     name=f"dm_{oh}_{k}", tag="dm")
                    # dmono[oh, k] is [ORDER, 128, 512]; partition dim is
                    # axis 1 of the slice -> per-partition rows of 512.
                    src = dm_d[oh, k].rearrange("m p o -> p m o")
                    nc.sync.dma_start(dm_t, src)
                    wt_t = dstr.tile([128, 512], F32R,
                                     name=f"wt_{oh}_{k}", tag="wt")
                    nc.sync.dma_start(wt_t, wt_d[oh, k])

                    for bh in range(2):
                        bsl = slice(512 * bh, 512 * (bh + 1))
                        tk = T[k][:, bsl]           # f32r [128, 512]
                        tkf = tk.bitcast(F32)
                        pw = pwp.tile([128, 7, 512], F32R,
                                      name=f"pw_{oh}_{k}_{bh}", tag="pw")
                        # slots: 0:t2 1:t3 2:t4 3:t5 4:t6 5:t7 6:t8
                        nc.scalar.activation(pw[:, 0, :], tkf, AF.Square)
                        nc.scalar.activation(pw[:, 2, :],
                                             pw[:, 0, :].bitcast(F32),
                                             AF.Square)
                        nc.scalar.activation(pw[:, 6, :],
                                             pw[:, 2, :].bitcast(F32),
                                             AF.Square)
                        nc.vector.tensor_mul(pw[:, 1, :],
                                             pw[:, 0, :].bitcast(F32), tkf)
                        nc.vector.tensor_mul(pw[:, 3, :],
                                             pw[:, 2, :].bitcast(F32), tkf)
                        nc.vector.tensor_mul(pw[:, 4, :],
                                             pw[:, 2, :].bitcast(F32),
                                             pw[:, 0, :].bitcast(F32))
                        nc.vector.tensor_mul(pw[:, 5, :],
                                             pw[:, 2, :].bitcast(F32),
                                             pw[:, 1, :].bitcast(F32))

                        for j4 in range(4):
                            j = 4 * bh + j4
                            jsl = slice(128 * j, 128 * (j + 1))
                            j4sl = slice(128 * j4, 128 * (j4 + 1))
                            if k == 0:
                                # K=1 ones-matmul injects the m=0 bias row;
                                # writes every element -> starts the group.
                                nc.tensor.matmul(ps[j], ones_t, v_t[:, osl],
                                                 start=True, stop=False)
                            nc.tensor.matmul(ps[j], SIL[k][:, jsl], wt_t,
                                             start=False, stop=False)
                            nc.tensor.matmul(ps[j], T[k][:, jsl],
                                             dm_t[:, 0, :],
                                             start=False, stop=False)
                            for m in range(2, ORDER + 1):
                                last = (k == N_K - 1 and m == ORDER)
                                nc.tensor.matmul(
                                    ps[j],
                                    pw[:, m - 2, j4sl],
                                    dm_t[:, m - 1, :],
                                    start=False, stop=last)
                # park this o-half
                for j in range(N_J):
                    nc.scalar.copy(z[j][:, osl], ps[j])

            # LayerNorm over o (=free dim) + silu, per row-tile.
            for j in range(N_J):
                st = stat.tile([128, 2, 6], F32, name=f"st_{j}", tag="st")
                nc.vector.bn_stats(st[:, 0, :], z[j][:, 0:512])
                nc.vector.bn_stats(st[:, 1, :], z[j][:, 512:1024])
                mv = stat.tile([128, 2], F32, name=f"mv_{j}", tag="mv")
                nc.vector.bn_aggr(mv, st)
                sd = stat.tile([128, 1], F32, name=f"sd_{j}", tag="sd")
                nc.scalar.activation(sd, mv[:, 1:2], AF.Sqrt, bias=eps_t)
                r = stat.tile([128, 1], F32, name=f"r_{j}", tag="r")
                nc.vector.reciprocal(r, sd)
                nb = stat.tile([128, 1], F32, name=f"nb_{j}", tag="nb")
                nc.vector.scalar_tensor_tensor(nb, mv[:, 0:1], -1.0, r,
                                               op0=ALU.mult, op1=ALU.mult)
                o_t = outp.tile([128, OUT_F], F32, name=f"o_{j}", tag="o")
                if general_ln:
                    zn = outp.tile([128, OUT_F], F32, name=f"zn_{j}",
                                   tag="zn")
                    nc.scalar.activation(zn, z[j], AF.Identity,
                                         bias=nb, scale=r)
                    nc.vector.tensor_mul(zn, zn, lnw_t)
                    nc.vector.tensor_add(zn, zn, lnb_t)
                    nc.scalar.activation(o_t, zn, AF.Silu)
                else:
                    nc.scalar.activation(o_t, z[j], AF.Silu,
                                         bias=nb, scale=r)
                nc.sync.dma_start(out_d[128 * j:128 * (j + 1), :], o_t)

    nc.compile()
    return nc


_PROG_CACHE = {}


def _get_program(general_ln):
    if general_ln not in _PROG_CACHE:
        _PROG_CACHE[general_ln] = _build_program(general_ln)
    return _PROG_CACHE[general_ln]


def _prep_shared(base_weights, jacobi_coeff, ln_weight, ln_bias, general_ln):
    M = _monomial_matrix()
    # D[:, :, m] = sum_d M[d, m] * C[:, :, d]
    D = np.einsum("dm,iod->iom", M, jacobi_coeff.astype(np.float64))

    v = D[:, :, 0].sum(axis=0).astype(np.float32).reshape(1, OUT_F)

    # dmono[oh, k, m-1, p, o] = D[128k+p, 512oh+o, m]
    Dp = np.transpose(D[:, :, 1:].astype(np.float32), (2, 0, 1))  # [8, in, out]
    Dp = Dp.reshape(ORDER, N_K, 128, N_OH, 512)
    dmono = np.ascontiguousarray(np.transpose(Dp, (3, 1, 0, 2, 4)))
    dmono = _q(dmono)

    # wtp[oh, k, p, o] = W[512oh+o, 128k+p]
    Wt = np.ascontiguousarray(base_weights.T)                # [in, out]
    Wt = Wt.reshape(N_K, 128, N_OH, 512)
    wtp = np.ascontiguousarray(np.transpose(Wt, (2, 0, 1, 3)))
    wtp = _q(wtp)

    shared = {
        "dmono": dmono,
        "wtp": wtp,
        "vrow": _q(v),
        "onerow": np.ones((1, 128), np.float32),
    }
    if general_ln:
        shared["lnw"] = np.ascontiguousarray(
            ln_weight.reshape(1, OUT_F).astype(np.float32))
        shared["lnb"] = np.ascontiguousarray(
            ln_bias.reshape(1, OUT_F).astype(np.float32))
    return shared


def kernel(x, base_weights, jacobi_coeff, ln_weight, ln_bias):
    x = np.asarray(x, np.float32).reshape(B_FULL, IN_F)
    base_weights = np.asarray(base_weights, np.float32)
    jacobi_coeff = np.asarray(jacobi_coeff, np.float32)
    ln_weight = np.asarray(ln_weight, np.float32)
    ln_bias = np.asarray(ln_bias, np.float32)

    general_ln = not (np.all(ln_weight == 1.0) and np.all(ln_bias == 0.0))

    nc = _get_program(general_ln)
    shared = _prep_shared(base_weights, jacobi_coeff, ln_weight, ln_bias,
                          general_ln)

    in_maps = []
    for c in range(N_CORES):
        xt = np.ascontiguousarray(
            x[B_CORE * c:B_CORE * (c + 1), :].T)     # [in, b_core]
        in_maps.append({"xt": xt, **shared})

    res = run_bass_kernel_spmd(nc, in_maps, core_ids=list(range(N_CORES)))
    out = np.concatenate([res.results[c]["out"] for c in range(N_CORES)],
                         axis=0)
    return out.astype(np.float32)


if __name__ == "__main__":
    rng = np.random.default_rng(1)
    demo = {
        "x": rng.standard_normal((B_FULL, IN_F)).astype(np.float32),
        "base_weights": rng.standard_normal((OUT_F, IN_F)).astype(np.float32) * 0.04,
        "jacobi_coeff": (rng.standard_normal((IN_F, OUT_F, ORDER + 1))
                         / (IN_F * (ORDER + 1))).astype(np.float32),
        "ln_weight": np.ones(OUT_F, np.float32),
        "ln_bias": np.zeros(OUT_F, np.float32),
    }
    o = kernel(**demo)
    print("kernel output:", o.shape, o.dtype, float(np.abs(o).mean()))


# revision 6
# speedup vs baseline: 277.3093x; 277.3093x over previous
"""JacobiKAN layer on 8 TRN2 NeuronCores — data-parallel Bass/Tile kernel.

  reference: out = silu(LN(silu(x) @ W.T + einsum('bid,iod->bo', jacobi(tanh x), C)))
  x [8192, 1024], W [1024, 1024], C [1024, 1024, 9]; order-8 Jacobi (a=b=1).

Strategy
  - Shard the token dim B=8192 across 8 cores (1024 rows each); weights
    replicated.  No collectives.
  - Reformulate the Jacobi einsum in the monomial basis:
        y = sum_m t^m @ D_m,   D_m = sum_d M[d,m] C[:,:,d]
    (M = Jacobi->monomial change of basis, host-precomputed).  The m=0 term
    is x-independent: a bias row v = sum_i D_0[i,:], injected with a K=1
    ones-matmul.  Device computes t^2..t^8 with 3 ScalarE squares + 4
    VectorE mults per tile — half the elementwise cost of the 2-op-per-
    degree Jacobi recurrence.
  - All matmuls in fp32r (fp32 with 12-bit mantissa, 1 PE cycle/row at
    N=512 — bf16 speed).  End-to-end scale-relative error vs the fp32
    reference is ~2e-4 (basis term dominates z, so the monomial
    amplification stays subdominant).
  - PSUM holds half the per-core output: two o-half passes, 8 banks of
    [128b, 512o] each; z parked in SBUF between passes; LayerNorm+SiLU
    fused at the end (bn_stats/bn_aggr + one ScalarE Silu with per-row
    scale/bias).
"""
import os
import sys
from contextlib import ExitStack

import numpy as np

for _p in ("/opt/trn_rl_repo",):
    if _p not in sys.path and os.path.isdir(_p):
        sys.path.append(_p)

import concourse.bacc as bacc
import concourse.mybir as mybir
import concourse.tile as tile
from concourse.bass_utils import run_bass_kernel_spmd
from neuronxcc.starfish.support.dtype import (
    static_cast_fp32_to_fp32r,
    static_cast_fp32r_to_fp32,
)

F32 = mybir.dt.float32
F32R = mybir.dt.float32r
AF = mybir.ActivationFunctionType
ALU = mybir.AluOpType

N_CORES = 8
B_FULL, IN_F, OUT_F, ORDER = 8192, 1024, 1024, 8
B_CORE = B_FULL // N_CORES          # 1024 rows per core
LN_EPS = 1e-5
N_K = IN_F // 128                   # 8 contraction chunks
N_J = B_CORE // 128                 # 8 output row-tiles per core
N_OH = 2                            # two 512-wide o halves (PSUM capacity)


def _q(a):
    """Round to fp32r (12-bit mantissa) — what the PE array consumes."""
    return static_cast_fp32r_to_fp32(
        static_cast_fp32_to_fp32r(np.ascontiguousarray(a, np.float32)))


def _monomial_matrix():
    """M[d, m]: P_d^{(1,1)}(t) = sum_m M[d,m] t^m (reference recurrence)."""
    a = b = 1.0
    M = np.zeros((ORDER + 1, ORDER + 1))
    M[0, 0] = 1.0
    M[1, 1] = (a + b + 2.0) / 2.0
    M[1, 0] = (a - b) / 2.0
    for i in range(2, ORDER + 1):
        th_k = (2 * i + a + b) * (2 * i + a + b - 1) / (2 * i * (i + a + b))
        th_k1 = ((2 * i + a + b - 1) * (a * a - b * b)
                 / (2 * i * (i + a + b) * (2 * i + a + b - 2)))
        th_k2 = ((i + a - 1) * (i + b - 1) * (2 * i + a + b)
                 / (i * (i + a + b) * (2 * i + a + b - 2)))
        M[i, 1:] += th_k * M[i - 1, :-1]
        M[i, :] += th_k1 * M[i - 1, :]
        M[i, :] -= th_k2 * M[i - 2, :]
    return M


def _build_program(general_ln, reps=1):
    """reps>1 wraps the whole body in a device-side For_i so wall-clock
    timing can amortize the PJRT dispatch overhead (test-only)."""
    import contextlib
    nc = bacc.Bacc("TRN2", target_bir_lowering=False, debug=False)

    xt_d = nc.dram_tensor("xt", [IN_F, B_CORE], F32, kind="ExternalInput").ap()
    dm_d = nc.dram_tensor("dmono", [N_OH, N_K, ORDER, 128, 512], F32R,
                          kind="ExternalInput").ap()
    wt_d = nc.dram_tensor("wtp", [N_OH, N_K, 128, 512], F32R,
                          kind="ExternalInput").ap()
    v_d = nc.dram_tensor("vrow", [1, OUT_F], F32R, kind="ExternalInput").ap()
    one_d = nc.dram_tensor("onerow", [1, 128], F32R, kind="ExternalInput").ap()
    if general_ln:
        lnw_d = nc.dram_tensor("lnw", [1, OUT_F], F32, kind="ExternalInput").ap()
        lnb_d = nc.dram_tensor("lnb", [1, OUT_F], F32, kind="ExternalInput").ap()
    out_d = nc.dram_tensor("out", [B_CORE, OUT_F], F32,
                           kind="ExternalOutput").ap()

    with tile.TileContext(nc) as tc:
        with ExitStack() as ctx:
            const = ctx.enter_context(tc.tile_pool(name="const", bufs=1))
            xload = ctx.enter_context(tc.tile_pool(name="xload", bufs=2))
            acts = ctx.enter_context(tc.tile_pool(name="acts", bufs=1))
            zpark = ctx.enter_context(tc.tile_pool(name="zpark", bufs=1))
            pwp = ctx.enter_context(tc.tile_pool(name="pwp", bufs=2))
            dstr = ctx.enter_context(tc.tile_pool(name="dstr", bufs=2))
            outp = ctx.enter_context(tc.tile_pool(name="outp", bufs=2))
            stat = ctx.enter_context(tc.tile_pool(name="stat", bufs=4))
            psum = ctx.enter_context(tc.tile_pool(name="psum", bufs=1,
                                                  space="PSUM"))

            ones_t = const.tile([1, 128], F32R)
            nc.sync.dma_start(ones_t, one_d)
            eps_t = const.tile([128, 1], F32)
            nc.vector.memset(eps_t, LN_EPS)
            v_t = const.tile([1, OUT_F], F32R)
            nc.sync.dma_start(v_t, v_d)
            if general_ln:
                import concourse.bass as bass
                lnw_t = const.tile([128, OUT_F], F32)
                nc.sync.dma_start(lnw_t, bass.AP(
                    tensor=lnw_d.tensor, offset=lnw_d.offset,
                    ap=[[0, 128]] + list(lnw_d.ap[1:])))
                lnb_t = const.tile([128, OUT_F], F32)
                nc.sync.dma_start(lnb_t, bass.AP(
                    tensor=lnb_d.tensor, offset=lnb_d.offset,
                    ap=[[0, 128]] + list(lnb_d.ap[1:])))

            loop_cm = (tc.For_i(0, reps, 1) if reps > 1
                       else contextlib.nullcontext())
            with loop_cm:
                _emit_body(nc, tc, xload, acts, zpark, pwp, dstr, outp, stat,
                           psum, xt_d, dm_d, wt_d, out_d, ones_t, v_t, eps_t,
                           lnw_t if general_ln else None,
                           lnb_t if general_ln else None)

    nc.compile()
    return nc


def _emit_body(nc, tc, xload, acts, zpark, pwp, dstr, outp, stat, psum,
               xt_d, dm_d, wt_d, out_d, ones_t, v_t, eps_t, lnw_t, lnb_t):
    general_ln = lnw_t is not None
    if True:
        if True:
            # T[k] = tanh(x.T chunk), SIL[k] = silu(x.T chunk); fp32r outs
            # (matmul lhsT producers must be fp32r-typed for the verifier).
            T = []
            SIL = []
            for k in range(N_K):
                xt_t = xload.tile([128, B_CORE], F32, name=f"xt_{k}",
                                  tag="xt")
                nc.sync.dma_start(xt_t, xt_d[128 * k:128 * (k + 1), :])
                t_t = acts.tile([128, B_CORE], F32R, name=f"tanh_{k}",
                                tag=f"tanh_{k}")
                nc.scalar.activation(t_t, xt_t, AF.Tanh)
                s_t = acts.tile([128, B_CORE], F32R, name=f"sil_{k}",
                                tag=f"sil_{k}")
                nc.scalar.activation(s_t, xt_t, AF.Silu)
                T.append(t_t)
                SIL.append(s_t)

            z = [zpark.tile([128, OUT_F], F32, name=f"z_{j}", tag=f"z_{j}")
                 for j in range(N_J)]

            for oh in range(N_OH):
                osl = slice(512 * oh, 512 * (oh + 1))
                ps = [psum.tile([128, 512], F32, name=f"ps_{oh}_{j}",
                                tag=f"ps_{j}") for j in range(N_J)]
                for k in range(N_K):
                    dm_t = dstr.tile([128, ORDER, 512], F32R,
                                     name=f"dm_{oh}_{k}", tag="dm")
                    # dmono[oh, k] is [ORDER, 128, 512]; partition dim is
                    # axis 1 of the slice -> per-partition rows of 512.
                    src = dm_d[oh, k].rearrange("m p o -> p m o")
                    nc.sync.dma_start(dm_t, src)
                    wt_t = dstr.tile([128, 512], F32R,
                                     name=f"wt_{oh}_{k}", tag="wt")
                    nc.sync.dma_start(wt_t, wt_d[oh, k])

                    for bh in range(2):
                        bsl = slice(512 * bh, 512 * (bh + 1))
                        tk = T[k][:, bsl]           # f32r [128, 512]
                        tkf = tk.bitcast(F32)
                        pw = pwp.tile([128, 7, 512], F32R,
                                      name=f"pw_{oh}_{k}_{bh}", tag="pw")
                        # slots: 0:t2 1:t3 2:t4 3:t5 4:t6 5:t7 6:t8
                        nc.scalar.activation(pw[:, 0, :], tkf, AF.Square)
                        nc.scalar.activation(pw[:, 2, :],
                                             pw[:, 0, :].bitcast(F32),
                                             AF.Square)
                        nc.scalar.activation(pw[:, 6, :],
                                             pw[:, 2, :].bitcast(F32),
                                             AF.Square)
                        nc.vector.tensor_mul(pw[:, 1, :],
                                             pw[:, 0, :].bitcast(F32), tkf)
                        nc.vector.tensor_mul(pw[:, 3, :],
                                             pw[:, 2, :].bitcast(F32), tkf)
                        nc.vector.tensor_mul(pw[:, 4, :],
                                             pw[:, 2, :].bitcast(F32),
                                             pw[:, 0, :].bitcast(F32))
                        nc.vector.tensor_mul(pw[:, 5, :],
                                             pw[:, 2, :].bitcast(F32),
                                             pw[:, 1, :].bitcast(F32))

                        for j4 in range(4):
                            j = 4 * bh + j4
                            jsl = slice(128 * j, 128 * (j + 1))
                            j4sl = slice(128 * j4, 128 * (j4 + 1))
                            if k == 0:
                                # K=1 ones-matmul injects the m=0 bias row;
                                # writes every element -> starts the group.
                                nc.tensor.matmul(ps[j], ones_t, v_t[:, osl],
                                                 start=True, stop=False)
                            nc.tensor.matmul(ps[j], SIL[k][:, jsl], wt_t,
                                             start=False, stop=False)
                            nc.tensor.matmul(ps[j], T[k][:, jsl],
                                             dm_t[:, 0, :],
                                             start=False, stop=False)
                            for m in range(2, ORDER + 1):
                                last = (k == N_K - 1 and m == ORDER)
                                nc.tensor.matmul(
                                    ps[j],
                                    pw[:, m - 2, j4sl],
                                    dm_t[:, m - 1, :],
                                    start=False, stop=last)
                # park this o-half
                for j in range(N_J):
                    nc.scalar.copy(z[j][:, osl], ps[j])

            # LayerNorm over o (=free dim) + silu, per row-tile.
            for j in range(N_J):
                st = stat.tile([128, 2, 6], F32, name=f"st_{j}", tag="st")
                nc.vector.bn_stats(st[:, 0, :], z[j][:, 0:512])
                nc.vector.bn_stats(st[:, 1, :], z[j][:, 512:1024])
                mv = stat.tile([128, 2], F32, name=f"mv_{j}", tag="mv")
                nc.vector.bn_aggr(mv, st)
                sd = stat.tile([128, 1], F32, name=f"sd_{j}", tag="sd")
                nc.scalar.activation(sd, mv[:, 1:2], AF.Sqrt, bias=eps_t)
                r = stat.tile([128, 1], F32, name=f"r_{j}", tag="r")
                nc.vector.reciprocal(r, sd)
                nb = stat.tile([128, 1], F32, name=f"nb_{j}", tag="nb")
                nc.vector.scalar_tensor_tensor(nb, mv[:, 0:1], -1.0, r,
                                               op0=ALU.mult, op1=ALU.mult)
                o_t = outp.tile([128, OUT_F], F32, name=f"o_{j}", tag="o")
                if general_ln:
                    zn = outp.tile([128, OUT_F], F32, name=f"zn_{j}",
                                   tag="zn")
                    nc.scalar.activation(zn, z[j], AF.Identity,
                                         bias=nb, scale=r)
                    nc.vector.tensor_mul(zn, zn, lnw_t)
                    nc.vector.tensor_add(zn, zn, lnb_t)
                    nc.scalar.activation(o_t, zn, AF.Silu)
                else:
                    nc.scalar.activation(o_t, z[j], AF.Silu,
                                         bias=nb, scale=r)
                nc.sync.dma_start(out_d[128 * j:128 * (j + 1), :], o_t)


_PROG_CACHE = {}


def _get_program(general_ln):
    if general_ln not in _PROG_CACHE:
        _PROG_CACHE[general_ln] = _build_program(general_ln)
    return _PROG_CACHE[general_ln]


def _prep_shared(base_weights, jacobi_coeff, ln_weight, ln_bias, general_ln):
    M = _monomial_matrix()
    # D[:, :, m] = sum_d M[d, m] * C[:, :, d]
    D = np.einsum("dm,iod->iom", M, jacobi_coeff.astype(np.float64))

    v = D[:, :, 0].sum(axis=0).astype(np.float32).reshape(1, OUT_F)

    # dmono[oh, k, m-1, p, o] = D[128k+p, 512oh+o, m]
    Dp = np.transpose(D[:, :, 1:].astype(np.float32), (2, 0, 1))  # [8, in, out]
    Dp = Dp.reshape(ORDER, N_K, 128, N_OH, 512)
    dmono = np.ascontiguousarray(np.transpose(Dp, (3, 1, 0, 2, 4)))
    dmono = _q(dmono)

    # wtp[oh, k, p, o] = W[512oh+o, 128k+p]
    Wt = np.ascontiguousarray(base_weights.T)                # [in, out]
    Wt = Wt.reshape(N_K, 128, N_OH, 512)
    wtp = np.ascontiguousarray(np.transpose(Wt, (2, 0, 1, 3)))
    wtp = _q(wtp)

    shared = {
        "dmono": dmono,
        "wtp": wtp,
        "vrow": _q(v),
        "onerow": np.ones((1, 128), np.float32),
    }
    if general_ln:
        shared["lnw"] = np.ascontiguousarray(
            ln_weight.reshape(1, OUT_F).astype(np.float32))
        shared["lnb"] = np.ascontiguousarray(
            ln_bias.reshape(1, OUT_F).astype(np.float32))
    return shared


def kernel(x, base_weights, jacobi_coeff, ln_weight, ln_bias):
    x = np.asarray(x, np.float32).reshape(B_FULL, IN_F)
    base_weights = np.asarray(base_weights, np.float32)
    jacobi_coeff = np.asarray(jacobi_coeff, np.float32)
    ln_weight = np.asarray(ln_weight, np.float32)
    ln_bias = np.asarray(ln_bias, np.float32)

    general_ln = not (np.all(ln_weight == 1.0) and np.all(ln_bias == 0.0))

    nc = _get_program(general_ln)
    shared = _prep_shared(base_weights, jacobi_coeff, ln_weight, ln_bias,
                          general_ln)

    in_maps = []
    for c in range(N_CORES):
        xt = np.ascontiguousarray(
            x[B_CORE * c:B_CORE * (c + 1), :].T)     # [in, b_core]
        in_maps.append({"xt": xt, **shared})

    res = run_bass_kernel_spmd(nc, in_maps, core_ids=list(range(N_CORES)))
    out = np.concatenate([res.results[c]["out"] for c in range(N_CORES)],
                         axis=0)
    return out.astype(np.float32)


if __name__ == "__main__":
    rng = np.random.default_rng(1)
    demo = {
        "x": rng.standard_normal((B_FULL, IN_F)).astype(np.float32),
        "base_weights": rng.standard_normal((OUT_F, IN_F)).astype(np.float32) * 0.04,
        "jacobi_coeff": (rng.standard_normal((IN_F, OUT_F, ORDER + 1))
                         / (IN_F * (ORDER + 1))).astype(np.float32),
        "ln_weight": np.ones(OUT_F, np.float32),
        "ln_bias": np.zeros(OUT_F, np.float32),
    }
    o = kernel(**demo)
    print("kernel output:", o.shape, o.dtype, float(np.abs(o).mean()))
